# revision 36
# baseline (speedup 1.0000x reference)
"""CLIP contrastive loss on 8 Trainium2 NeuronCores.

Strategy (data parallel over rows, per the sharding hint):
  - Both feature matrices are row-sharded 8 x [2048, 512].
  - Each core PE-transposes its shards to D-major, folds sqrt(1/temp) in,
    casts to fp8-e4m3, then AllGathers both transposed matrices.
  - Pass 1 (image rows): L = img_shard @ txt_all^T via fp8 DoubleRow
    matmuls into [128, 1024] PSUM tiles (4 pipeline slots); per tile:
    VectorE negated rowmax, then one fused ScalarE exp(x - max) with
    free-dim sum accumulation (accum_out). Block s=0 of each pass uses
    the SBUF-resident own shard so compute overlaps the AllGathers;
    s>0 reads block (rank+s)%8 via a rank-rotated dynamic DMA.
  - Pass 2 (text rows): same with roles swapped.
  - diag terms <img_i, txt_i> are computed in fp32 (GpSimd mul + VectorE
    reduce) from the raw inputs, so label logits are exact.
  - Each core outputs its per-row-group stats [128, 4*256+1]; the host
    merges them into the scalar loss in f64. All O(N^2) work is on-device.
"""
import sys

if "/opt/trn_rl_repo" not in sys.path:
    sys.path.insert(0, "/opt/trn_rl_repo")

import numpy as np

from concourse import bacc, bass, mybir, tile
from concourse.bass_utils import run_bass_kernel_spmd
from concourse.masks import make_identity

SCALE = 1.0 / 0.07
N = 16384
D = 512
NCORES = 8
LN = N // NCORES          # 2048 local rows
P = 128
R = LN // P               # 16 row tiles per core
KC = D // P               # 4 contraction chunks
CH = 512                  # matmul moving free dim (one PSUM bank)
CPB = LN // CH            # 4 chunks per block
NB = NCORES               # 8 column blocks (one per source core)
GW = 1024                 # stat-group width (2 PSUM banks -> 4 pipeline slots)
GPB = LN // GW            # 2 stat groups per block
NG = NB * GPB             # 16 stat groups per row tile
SQS = SCALE ** 0.5        # sqrt(scale), folded into both operands

F32 = mybir.dt.float32
BF16 = mybir.dt.bfloat16
FP8 = mybir.dt.float8e4


def build():
    nc = bacc.Bacc(None, target_bir_lowering=False, debug=False, num_devices=NCORES)

    img_ext = nc.dram_tensor("image_features", [LN, D], F32, kind="ExternalInput")
    txt_ext = nc.dram_tensor("text_features", [LN, D], F32, kind="ExternalInput")
    out_ext = nc.dram_tensor("out", [P, 4 * R * NG + 1], F32, kind="ExternalOutput")

    with tile.TileContext(nc) as tc:
        with (
            tc.tile_pool(name="dram", bufs=1, space="DRAM") as dram,
            tc.tile_pool(name="const", bufs=1) as const,
            tc.tile_pool(name="persist", bufs=1) as persist,
            tc.tile_pool(name="stats", bufs=1) as stats,
        ):
            itb = dram.tile([D, LN], FP8)
            ttb = dram.tile([D, LN], FP8)
            itg = dram.tile([NCORES * D, LN], FP8, addr_space="Shared")
            ttg = dram.tile([NCORES * D, LN], FP8, addr_space="Shared")

            ident = const.tile([P, P], F32)
            make_identity(nc, ident)

            # persistent D-major fp8 shards: [p = d % 128, dk, i]
            imgT = persist.tile([P, KC, LN], FP8)
            txtT = persist.tile([P, KC, LN], FP8)

            # per (row-tile, group) stats, col = r * NG + g; mx holds -max
            mx0 = stats.tile([P, R * NG], F32)
            ss0 = stats.tile([P, R * NG], F32)
            mx1 = stats.tile([P, R * NG], F32)
            ss1 = stats.tile([P, R * NG], F32)
            diag_pp = stats.tile([P, 1], F32)

            # ---------------- setup: load, diag, transpose, gather ----------
            with (
                tc.tile_pool(name="setup", bufs=1) as setup,
                tc.tile_pool(name="tpsum", bufs=4, space="PSUM") as tpsum,
            ):
                img_sb = setup.tile([P, R, D], F32)
                txt_sb = setup.tile([P, R, D], F32)
                RQ = R // 4
                for q in range(4):
                    nc.sync.dma_start(
                        txt_sb[:, q * RQ:(q + 1) * RQ, :],
                        txt_ext[q * RQ * P:(q + 1) * RQ * P, :].rearrange(
                            "(r p) d -> p r d", p=P
                        ),
                    )
                for q in range(4):
                    nc.sync.dma_start(
                        img_sb[:, q * RQ:(q + 1) * RQ, :],
                        img_ext[q * RQ * P:(q + 1) * RQ * P, :].rearrange(
                            "(r p) d -> p r d", p=P
                        ),
                    )

                # diag partial: sum_d img[i,d]*txt[i,d] (unscaled fp32),
                # in quarters so the GpSimd muls start as input-DMA quarters
                # land and the DVE reduces fill the pre-matmul idle window.
                dtmp = setup.tile([P, R, D], F32)
                dsum = setup.tile([P, R], F32)
                for q in range(4):
                    rs = slice(q * RQ, (q + 1) * RQ)
                    nc.gpsimd.tensor_mul(
                        dtmp[:, rs, :], img_sb[:, rs, :], txt_sb[:, rs, :]
                    )
                    nc.vector.reduce_sum(
                        dsum[:, rs], dtmp[:, rs, :], axis=mybir.AxisListType.X
                    )
                nc.vector.reduce_sum(diag_pp[:], dsum[:], axis=mybir.AxisListType.X)

                # text first so its AllGather is issued as early as possible
                for src, dstT in ((txt_sb, txtT), (img_sb, imgT)):
                    for r in range(R):
                        tp = tpsum.tile([P, KC, P], F32, name="tp")
                        for dk in range(KC):
                            nc.tensor.transpose(
                                tp[:, dk, :],
                                src[:, r, dk * P:(dk + 1) * P],
                                ident[:],
                            )
                        if r % 2 == 0:
                            nc.scalar.activation(
                                dstT[:, :, r * P:(r + 1) * P],
                                tp[:],
                                mybir.ActivationFunctionType.Copy,
                                scale=SQS,
                            )
                        else:
                            nc.vector.tensor_scalar_mul(
                                dstT[:, :, r * P:(r + 1) * P], tp[:], SQS
                            )
                    if dstT is txtT:
                        nc.sync.dma_start(
                            ttb[:].rearrange("(dk p) i -> p dk i", p=P), txtT[:]
                        )
                        nc.gpsimd.collective_compute(
                            "AllGather",
                            mybir.AluOpType.bypass,
                            replica_groups=[list(range(NCORES))],
                            ins=[ttb[:].opt()],
                            outs=[ttg[:].opt()],
                        )
                    else:
                        nc.sync.dma_start(
                            itb[:].rearrange("(dk p) i -> p dk i", p=P), imgT[:]
                        )
                        nc.gpsimd.collective_compute(
                            "AllGather",
                            mybir.AluOpType.bypass,
                            replica_groups=[list(range(NCORES))],
                            ins=[itb[:].opt()],
                            outs=[itg[:].opt()],
                        )


            # ---------------- main passes ----------------------------------
            with (
                tc.tile_pool(name="stream", bufs=3) as stream,
                tc.tile_pool(name="mpsum", bufs=4, space="PSUM") as mpsum,
                tc.tile_pool(name="small", bufs=4) as small,
            ):
                # rank of this core: block s=0 of each pass uses the
                # SBUF-resident own shard while the AllGathers are in flight;
                # s>0 reads block (rank+s)%8 from the gathered buffer.
                rank = nc.sync.snap(
                    nc.sync.cc_rank(replica_groups=[list(range(NCORES))]),
                    min_val=0,
                    max_val=NCORES - 1,
                )
                cfgs = [
                    (imgT, ttg, mx0, ss0, txtT),
                    (txtT, itg, mx1, ss1, imgT),
                ]

                def emit_block(pi, s):
                    lhsT, gsrc, mx, ss, own = cfgs[pi]
                    if s == 0:
                        rhs = own
                    else:
                        rhs = stream.tile([P, KC, LN], FP8, name="rhs", tag="rhs")
                        bb = (rank + s) % NCORES
                        nc.sync.dma_start(
                            rhs[:],
                            gsrc[bass.ds(bb * D, D), :].rearrange(
                                "(dk p) j -> p dk j", p=P
                            ),
                        )
                    for r in range(R):
                        for h in range(GPB):
                            pt = mpsum.tile([P, GW], F32, name="pt", tag="pt")
                            for c in range(GW // CH):
                                cc = h * (GW // CH) + c
                                for k in range(0, KC, 2):
                                    nc.tensor.matmul(
                                        pt[:, c * CH:(c + 1) * CH],
                                        lhsT[:, k:k + 2, r * P:(r + 1) * P],
                                        rhs[:, k:k + 2, cc * CH:(cc + 1) * CH],
                                        start=(k == 0),
                                        stop=(k == KC - 2),
                                        perf_mode=mybir.MatmulPerfMode.DoubleRow,
                                    )
                            col = r * NG + s * GPB + h
                            # nmx = -max(psum); psum already holds SCALE*logits
                            rmx = nc.vector.reduce_max(
                                mx[:, col:col + 1],
                                pt[:],
                                axis=mybir.AxisListType.X,
                                negate=True,
                            )

                            nc.scalar.activation(
                                pt[:],
                                pt[:],
                                mybir.ActivationFunctionType.Exp,
                                bias=mx[:, col:col + 1],
                                accum_out=ss[:, col:col + 1],
                            )

                SG = R * NG
                emit_block(0, 0)
                emit_block(1, 0)
                for s in range(1, NB):
                    emit_block(0, s)
                # pass-1 stats stream out while pass 2 computes
                nc.sync.dma_start(out_ext[:, 0 * SG:1 * SG], mx0[:])
                nc.sync.dma_start(out_ext[:, 1 * SG:2 * SG], ss0[:])
                for s in range(1, NB):
                    emit_block(1, s)
                nc.sync.dma_start(out_ext[:, 2 * SG:3 * SG], mx1[:])
                nc.sync.dma_start(out_ext[:, 3 * SG:4 * SG], ss1[:])
                nc.sync.dma_start(out_ext[:, 4 * SG:4 * SG + 1], diag_pp[:])


    nc.compile()
    return nc


_NC_CACHE = None


def _get_nc():
    global _NC_CACHE
    if _NC_CACHE is None:
        _NC_CACHE = build()
    return _NC_CACHE


def kernel(image_features: np.ndarray, text_features: np.ndarray) -> np.ndarray:
    img = np.ascontiguousarray(np.asarray(image_features, dtype=np.float32))
    txt = np.ascontiguousarray(np.asarray(text_features, dtype=np.float32))
    assert img.shape == (N, D) and txt.shape == (N, D)

    nc = _get_nc()
    in_maps = [
        {
            "image_features": img[i * LN:(i + 1) * LN],
            "text_features": txt[i * LN:(i + 1) * LN],
        }
        for i in range(NCORES)
    ]
    res = run_bass_kernel_spmd(nc, in_maps, core_ids=list(range(NCORES)))

    # host-side merge of per-group stats (f64): for each local row,
    # lse = M + ln(sum_g ss_g * exp(mx_g - M)), M = max_g mx_g = -min_g nmx_g
    SG = R * NG
    lse_tot = 0.0
    diag = 0.0
    for om in res.results:
        part = om["out"].astype(np.float64)
        for o in range(2):
            nmx = part[:, (2 * o) * SG:(2 * o + 1) * SG].reshape(P, R, NG)
            ss = part[:, (2 * o + 1) * SG:(2 * o + 2) * SG].reshape(P, R, NG)
            nm = nmx.min(axis=2, keepdims=True)
            stot = (ss * np.exp(nm - nmx)).sum(axis=2)
            lse_tot += (-nm[:, :, 0] + np.log(stot)).sum()
        diag += part[:, 4 * SG].sum()

    loss = (lse_tot - 2.0 * SCALE * diag) / (2.0 * N)
    return np.float32(loss)


if __name__ == "__main__":
    rng = np.random.default_rng(0)
    a = rng.standard_normal((N, D)).astype(np.float32)
    b = rng.standard_normal((N, D)).astype(np.float32)
    print("loss:", kernel(a, b))
